# revision 1
# baseline (speedup 1.0000x reference)
"""MoE routing kernel for Trainium2, expert-parallel across 8 NeuronCores.

Sharding: experts are sorted by dispatch count and dealt round-robin so that
slot position j on every core has the same padded capacity caps[j] (baked into
the SPMD program). The gate/top-k/dispatch runs on host as part of sharding;
each core receives its experts' dispatched token rows in a partition-major
layout (one large contiguous per-partition block per tensor, so every
dma_start moves 4-16KB per partition), its expert weights, and a slice of
tokens for the (replicated-weight) shared expert. Device computes the grouped
SwiGLU expert GEMMs + shared expert with slots always on the moving dim (no
partial-tile matmul waste) and SwiGLU pairs processed two-at-a-time so the
silu/mul chain overlaps the next pair's matmuls. Host gathers per-slot
outputs and does the weighted combine (unshard).
"""

import numpy as np
import ml_dtypes

import bass_rust
import concourse.bass as bass
import concourse.mybir as mybir
from concourse.tile import TileContext
from concourse.vector_clock import ScopedClock
from concourse.bass_utils import run_bass_kernel_spmd

B, T, C = 2, 2048, 2048
N = B * T
E, H, HS = 64, 256, 512
TOPK = 6
NCORES = 8
ELOC = E // NCORES  # 8 experts per core
NLOC = N // NCORES  # 512 tokens per core for the shared expert
BF16 = mybir.dt.bfloat16
F32 = mybir.dt.float32
P = 128
KC = C // P  # 16 contraction chunks over C

_BF16_NP = ml_dtypes.bfloat16

# shared-up pair order: pair i computes y chunk m=i and gate chunk m=4+i of
# the 2*HS=1024 up-projection columns
_MPAIR = [0, 4, 1, 5, 2, 6, 3, 7]


# --------------------------------------------------------------------------
# Tile tail-drain fix: this walrus build allows at most one semaphore wait per
# instruction (none on Drain). Tile's end-of-context drain carries the whole
# global clock; emit a chain of single-wait NOPs on SP instead.
# --------------------------------------------------------------------------
def _patched_drain_and_barrier(self, tick_clock, wait_clock):
    carrier = self.nc.sync.nop(nofuse=True, hint="tail_wait_0")
    wait_clock.add_sem_waits(carrier.ins, ScopedClock({None: tick_clock.global_clock}))
    si = carrier.ins.sync_info
    waits = list(si.on_wait) if si else []
    upds = list(si.on_update) if si else []
    carrier.ins.sync_info = bass_rust.SyncInfo(on_wait=waits[:1], on_update=upds)
    for i, w in enumerate(waits[1:]):
        n2 = self.nc.sync.nop(nofuse=True, hint=f"tail_wait_{i + 1}")
        n2.ins.sync_info = bass_rust.SyncInfo(on_wait=[w], on_update=[])

    self.nc.sync.drain()
    self.nc.all_engine_barrier()
    assert self.sems is not None
    popped = self.nc._tile_sem_poison_stack.pop()
    assert popped is self._sem_poison
    # sems still cleared for NEFF re-execution, but the trailing all-engine
    # barrier is redundant: engines are already drained and barriered, and
    # the gpsimd clear halts on its own before the NEFF ends
    self.nc.clear_and_free_semaphores(list(self.sems.allocated().values()))


_orig_add_instruction = TileContext._add_instruction


def _patched_add_instruction(self, inst):
    si = getattr(inst, "sync_info", None)
    if si is not None and len(si.on_wait) > 1:
        waits = list(si.on_wait)
        for w in waits[:-1]:
            nop = mybir.InstNoOp(
                name=self.nc.get_next_instruction_name(), ins=[], outs=[])
            nop.engine = inst.engine
            nop.sync_info = bass_rust.SyncInfo(on_wait=[w], on_update=[])
            _orig_add_instruction(self, nop)
        inst.sync_info = bass_rust.SyncInfo(
            on_wait=[waits[-1]], on_update=list(si.on_update))
    _orig_add_instruction(self, inst)


def _install_drain_fix():
    if getattr(TileContext, "_drain_fix_installed", False):
        return
    TileContext._drain_and_barrier = _patched_drain_and_barrier
    TileContext._add_instruction = _patched_add_instruction
    TileContext._drain_fix_installed = True


# --------------------------------------------------------------------------
# Device kernel
# --------------------------------------------------------------------------
_BUILD_CACHE = {}


def _build(caps):
    """Per-core Bass program; caps[j] = padded capacity of slot position j."""
    _install_drain_fix()
    nc = bass.Bass()

    offs = [0]
    for cp in caps:
        offs.append(offs[-1] + cp)
    S = offs[-1]

    # all HBM tensors are partition-major: [128, X] with large contiguous
    # per-partition runs so DMA packets are 4-16KB
    xdh = nc.declare_dram_parameter("xdh", [P, KC * S], BF16, isOutput=False)
    wuh = nc.declare_dram_parameter("wuh", [P, ELOC * KC * 2 * H], BF16, isOutput=False)
    wdh = nc.declare_dram_parameter("wdh", [P, ELOC * 2 * C], BF16, isOutput=False)
    xsh = nc.declare_dram_parameter("xsh", [P, KC * NLOC], BF16, isOutput=False)
    wsuh = nc.declare_dram_parameter("wsuh", [P, KC * 2 * HS], BF16, isOutput=False)
    wsdh = nc.declare_dram_parameter("wsdh", [P, 4 * C], BF16, isOutput=False)
    yrh = nc.declare_dram_parameter("yrh", [P, KC * S], BF16, isOutput=True)
    ysh = nc.declare_dram_parameter("ysh", [NLOC, C], BF16, isOutput=True)

    with TileContext(nc) as tc:
        with (
            tc.tile_pool(name="xsg_sb", bufs=4) as xsg_pool,
            tc.tile_pool(name="wsug0_sb", bufs=4) as wsug0_pool,
            tc.tile_pool(name="wsug_sb", bufs=2) as wsug_pool,
            tc.tile_pool(name="wsd_sb", bufs=1) as wsd_pool,
            tc.tile_pool(name="hsh_sb", bufs=1) as hsh_pool,
            tc.tile_pool(name="osh_sb", bufs=3) as osh_pool,
            tc.tile_pool(name="xd_sb", bufs=2) as xd_pool,
            tc.tile_pool(name="wu_sb", bufs=2) as wu_pool,
            tc.tile_pool(name="wd_sb", bufs=2) as wd_pool,
            tc.tile_pool(name="yo_sb", bufs=2) as yo_pool,
            tc.tile_pool(name="h_sb", bufs=2) as h_pool,
            tc.tile_pool(name="sg_sb", bufs=2) as sg_pool,
            tc.tile_pool(name="pu", bufs=4, space="PSUM") as pu_pool,
            tc.tile_pool(name="pd", bufs=2, space="PSUM") as pd_pool,
        ):
            xd_t = [None] * ELOC
            wu_t = [None] * ELOC
            wd_t = [None] * ELOC
            h_t = [None] * ELOC

            def load_expert_xd(j):
                cap = caps[j]
                base = KC * offs[j]
                xd_t[j] = xd_pool.tile([P, KC * cap], BF16, tag="xd",
                                       name=f"xd_{j}")
                nc.sync.dma_start(
                    out=xd_t[j][:], in_=xdh[:, base:base + KC * cap])

            def load_expert_wu(j):
                wu_t[j] = wu_pool.tile([P, KC * 512], BF16, tag="wu",
                                       name=f"wu_{j}")
                nc.sync.dma_start(
                    out=wu_t[j][:],
                    in_=wuh[:, j * KC * 512:(j + 1) * KC * 512])

            def load_expert_wd(j, engine=None):
                wd_t[j] = wd_pool.tile([P, 2 * C], BF16, tag="wd",
                                       name=f"wd_{j}")
                (engine or nc.gpsimd).dma_start(
                    out=wd_t[j][:], in_=wdh[:, j * 2 * C:(j + 1) * 2 * C])

            def up_expert(j):
                # up in two m-half passes; half hf computes gate chunk m=hf
                # and v chunk m=2+hf -> h chunk hf; half 0's silu/mul
                # overlaps half 1's matmuls
                cap = caps[j]
                xd, wu = xd_t[j], wu_t[j]
                assert cap <= 512
                w_ = cap
                h = h_pool.tile([P, 2 * w_], BF16, tag="h", name=f"h_{j}")
                for hf in range(2):
                    p_g = pu_pool.tile([P, w_], F32, space="PSUM",
                                       tag="pu", name=f"pug_{j}_{hf}")
                    p_v = pu_pool.tile([P, w_], F32, space="PSUM",
                                       tag="pu", name=f"puv_{j}_{hf}")
                    for k in range(KC):
                        lb = k * 512
                        rhs = xd[:, k * cap:k * cap + w_]
                        nc.tensor.matmul(
                            out=p_g[:],
                            lhsT=wu[:, lb + hf * P:lb + (hf + 1) * P],
                            rhs=rhs,
                            start=(k == 0), stop=(k == KC - 1))
                        nc.tensor.matmul(
                            out=p_v[:],
                            lhsT=wu[:, lb + (2 + hf) * P:lb + (3 + hf) * P],
                            rhs=rhs,
                            start=(k == 0), stop=(k == KC - 1))
                    sg = sg_pool.tile([P, w_], F32, tag="sg",
                                      name=f"sg_{j}_{hf}")
                    nc.scalar.activation(sg[:], p_g[:],
                                         mybir.ActivationFunctionType.Silu)
                    nc.vector.tensor_mul(
                        h[:, hf * w_:(hf + 1) * w_], sg[:], p_v[:])
                h_t[j] = h

            def down_expert(j, store_quarters=False):
                # down: out = [128 of C, cap slots]; two C chunks share a
                # 2-bank psum tile (sub-outputs bank-aligned at col 0 / 512)
                cap = caps[j]
                base = KC * offs[j]
                wd, h = wd_t[j], h_t[j]
                w_ = cap
                yo = yo_pool.tile([P, KC * cap], BF16, tag="yo",
                                  name=f"yo_{j}")
                for ccp in range(8):
                    pd = pd_pool.tile([P, 1024], F32, space="PSUM",
                                      tag="pd", name=f"pd_{j}_{ccp}")
                    for sub in range(2):
                        cc = 2 * ccp + sub
                        for kh in range(2):
                            nc.tensor.matmul(
                                out=pd[:, sub * 512:sub * 512 + w_],
                                lhsT=wd[:, kh * C + cc * P:
                                        kh * C + (cc + 1) * P],
                                rhs=h[:, kh * w_:(kh + 1) * w_],
                                start=(kh == 0), stop=(kh == 1))
                    if w_ == 512:
                        dsts = [yo[:, 2 * ccp * cap:(2 * ccp + 2) * cap]]
                        srcs = [pd[:]]
                    else:
                        dsts = [yo[:, (2 * ccp + s) * cap:
                                   (2 * ccp + s) * cap + w_]
                                for s in range(2)]
                        srcs = [pd[:, s * 512:s * 512 + w_] for s in range(2)]
                    for dst, srcp in zip(dsts, srcs):
                        if ccp % 4 == 3:
                            nc.scalar.copy(dst, srcp)
                        else:
                            nc.vector.tensor_copy(out=dst, in_=srcp)
                    if store_quarters:
                        if ccp % 2 == 1:
                            lo = (ccp - 1) * 2 * cap
                            nc.scalar.dma_start(
                                out=yrh[:, base + lo:base + lo + 4 * cap],
                                in_=yo[:, lo:lo + 4 * cap])
                    elif ccp in (3, 7):
                        lo = 0 if ccp == 3 else 8 * cap
                        nc.scalar.dma_start(
                            out=yrh[:, base + lo:base + lo + 8 * cap],
                            in_=yo[:, lo:lo + 8 * cap])

            # ------------- loads: one sync queue, consumption order ---------
            # shared-up pass-0 inputs quartered so the first matmul's inputs
            # land as early as possible
            # wsu pair-0 and xs interleaved in quarters so pass-0's
            # matmuls are fed just-in-time as each k-group lands
            wsu0_q = []
            xs_q = []
            for qq in range(4):
                t = wsug0_pool.tile([P, 4 * 256], BF16, tag="wsug0",
                                    name=f"wsu_0_{qq}")
                nc.sync.dma_start(
                    out=t[:], in_=wsuh[:, qq * 1024:(qq + 1) * 1024])
                wsu0_q.append(t)
                t2 = xsg_pool.tile([P, 4 * NLOC], BF16, tag="xsg",
                                   name=f"xs_{qq}")
                nc.sync.dma_start(
                    out=t2[:], in_=xsh[:, qq * 4 * NLOC:(qq + 1) * 4 * NLOC])
                xs_q.append(t2)
            wsu_p = [None] * 4
            for pr in range(1, 4):
                t = wsug_pool.tile([P, KC * 256], BF16, tag="wsug",
                                   name=f"wsu_{pr}")
                nc.sync.dma_start(
                    out=t[:], in_=wsuh[:, pr * 4096:(pr + 1) * 4096])
                wsu_p[pr] = t
            load_expert_xd(0)
            load_expert_wu(0)
            wsd_t = wsd_pool.tile([P, 4 * C], BF16, tag="wsd")
            nc.sync.dma_start(out=wsd_t[:], in_=wsdh[:])
            load_expert_wd(0, engine=nc.sync)
            load_expert_xd(1)
            load_expert_wu(1)
            load_expert_wd(1, engine=nc.sync)

            # ------------- PE warm-up: dummy matmuls with no input deps -----
            # the HAM clock gate starts at 1.2 GHz and needs ~3.4us of
            # sustained PE activity to release; burn the DMA-wait window
            # (scratch = a corner of hsh, whose real writes come much later)
            hsh = hsh_pool.tile([P, 4 * NLOC], BF16, tag="hsh")
            nc.vector.memset(hsh[:, 0:512], 0.0)
            pwarm = pd_pool.tile([P, 1024], F32, space="PSUM", tag="pd",
                                 name="pwarm")
            for wi in range(16):
                nc.tensor.matmul(out=pwarm[:, 0:512], lhsT=hsh[:, 0:P],
                                 rhs=hsh[:, 0:512], start=True, stop=True)

            # ------------- shared up: 4 pair passes x 2 psums ---------------
            # hsh holds h = silu(g_s)*y_s as 4 chunks of [128, 512] columns;
            # each pair's silu/mul overlaps the next pair's matmuls
            for pr in range(4):
                ps_y = pu_pool.tile([P, NLOC], F32, space="PSUM", tag="pu",
                                    name=f"ps_y{pr}")
                ps_g = pu_pool.tile([P, NLOC], F32, space="PSUM", tag="pu",
                                    name=f"ps_g{pr}")
                for k in range(KC):
                    if pr == 0:
                        wt, lo = wsu0_q[k // 4], (k % 4) * 256
                    else:
                        wt, lo = wsu_p[pr], k * 256
                    rhs = xs_q[k // 4][:, (k % 4) * NLOC:(k % 4 + 1) * NLOC]
                    nc.tensor.matmul(
                        out=ps_y[:], lhsT=wt[:, lo:lo + P], rhs=rhs,
                        start=(k == 0), stop=(k == KC - 1))
                    nc.tensor.matmul(
                        out=ps_g[:], lhsT=wt[:, lo + P:lo + 2 * P], rhs=rhs,
                        start=(k == 0), stop=(k == KC - 1))
                sg = sg_pool.tile([P, NLOC], F32, tag="sg", name=f"sgs_{pr}")
                nc.scalar.activation(sg[:], ps_g[:],
                                     mybir.ActivationFunctionType.Silu)
                nc.vector.tensor_mul(
                    hsh[:, pr * NLOC:(pr + 1) * NLOC], sg[:], ps_y[:])

            # expert 0's up runs here: it gives the tensor engine work while
            # the shared-down inputs and the expert pipeline loads stream in
            up_expert(0)

            # ------------- shared down: tokens as out rows ------------------
            # two C-column chunks share one 2-bank psum tile; copies all on
            # vector (the scalar ACT copy is ~2x slower and would pace the
            # pd-slot rotation)
            for mt in range(4):
                osh = osh_pool.tile([P, C], BF16, tag="osh", name=f"osh_{mt}")
                for ncp in range(2):
                    pd = pd_pool.tile([P, 1024], F32, space="PSUM", tag="pd",
                                      name=f"pds_{mt}_{ncp}")
                    for sub in range(2):
                        ncc = 2 * ncp + sub
                        for kh in range(4):
                            nc.tensor.matmul(
                                out=pd[:, sub * 512:(sub + 1) * 512],
                                lhsT=hsh[:, kh * NLOC + mt * P:
                                         kh * NLOC + (mt + 1) * P],
                                rhs=wsd_t[:, kh * C + ncc * 512:
                                          kh * C + (ncc + 1) * 512],
                                start=(kh == 0), stop=(kh == 3))
                    nc.vector.tensor_copy(
                        out=osh[:, ncp * 1024:(ncp + 1) * 1024], in_=pd[:])
                nc.gpsimd.dma_start(out=ysh[mt * P:(mt + 1) * P, :], in_=osh[:])

            # ------------- routed experts (software-pipelined) --------------
            # up_{j+1} is issued before down_j so the pd psum->sbuf copies of
            # down_j drain behind up_{j+1}'s matmuls
            up_expert(1)
            down_expert(0)
            for j in range(2, ELOC):
                load_expert_xd(j)
                load_expert_wu(j)
                load_expert_wd(j)
                up_expert(j)
                down_expert(j - 1)
            down_expert(ELOC - 1, store_quarters=True)
    return nc


# --------------------------------------------------------------------------
# Host wrapper
# --------------------------------------------------------------------------
def _pm(a, nchunk):
    """[nchunk*128, X] row-major -> partition-major [128, nchunk*X]."""
    x = a.shape[1]
    return np.ascontiguousarray(
        a.reshape(nchunk, P, x).transpose(1, 0, 2)).reshape(P, nchunk * x)


def kernel(x, w_gate, w_shared_up, w_shared_down, w_up, w_down):
    x_flat = x.reshape(-1, C)

    # ---- gate: sigmoid scores, top-6, normalized weights (f64 for a stable
    # ordering; ties in the fp32 reference are measure-zero) ----
    logits = x_flat.astype(np.float64) @ w_gate.astype(np.float64)
    scores = 1.0 / (1.0 + np.exp(-logits))
    topk_idx = np.argsort(-scores, axis=-1, kind="stable")[:, :TOPK]
    w = np.take_along_axis(scores, topk_idx, axis=-1)
    w = w / w.sum(-1, keepdims=True)

    # ---- dispatch positions (stable within each expert, slot-major order) --
    flat_e = topk_idx.reshape(-1)
    order = np.argsort(flat_e, kind="stable")
    sorted_e = flat_e[order]
    group_start = np.searchsorted(sorted_e, np.arange(E))
    counts = np.bincount(flat_e, minlength=E)

    token_of_slot = np.arange(N * TOPK) // TOPK
    expert_slots = []   # flat (token,k) slot ids, dispatch order, per expert
    expert_tokens = []
    for e in range(E):
        slots = order[group_start[e]: group_start[e] + counts[e]]
        expert_slots.append(slots)
        expert_tokens.append(token_of_slot[slots])

    # ---- balanced expert->core assignment: sort by count desc, deal 8 at a
    # time; slot position j has the same padded cap on every core ----
    ranks = np.argsort(-counts, kind="stable")
    expert_of = [[int(ranks[8 * j + c]) for j in range(ELOC)]
                 for c in range(NCORES)]
    caps = tuple(
        max(8, int(-(-int(counts[ranks[8 * j]]) // 8) * 8)) for j in range(ELOC))
    offs = [0]
    for cp in caps:
        offs.append(offs[-1] + cp)

    # ---- build per-core inputs (partition-major bf16) ----
    xT_bf = np.ascontiguousarray(x_flat.T).astype(_BF16_NP)  # [C, N]
    wsu_f = w_shared_up.astype(_BF16_NP)
    wsd_f = w_shared_down.astype(_BF16_NP)

    # shared-up weights in pair/k-major order (see _MPAIR)
    wr = wsu_f.reshape(KC, P, 8, P)[:, :, _MPAIR, :]        # [k, p, 8, 128]
    wsuh = np.ascontiguousarray(
        wr.reshape(KC, P, 4, 2 * P).transpose(1, 2, 0, 3)).reshape(P, KC * 2 * HS)
    wsdh = _pm(wsd_f, 4)

    in_maps = []
    for c in range(NCORES):
        xd_blocks = []
        wu_blocks = []
        wd_blocks = []
        for j in range(ELOC):
            e = expert_of[c][j]
            tok = expert_tokens[e]
            n = len(tok)
            blk = np.zeros((P, KC, caps[j]), dtype=_BF16_NP)
            blk[:, :, :n] = xT_bf[:, tok].reshape(KC, P, n).transpose(1, 0, 2)
            xd_blocks.append(blk.reshape(P, -1))
            wu_blocks.append(_pm(w_up[e].astype(_BF16_NP), KC))
            wd_blocks.append(_pm(w_down[e].astype(_BF16_NP), 2))
        xsh = _pm(np.ascontiguousarray(
            xT_bf[:, c * NLOC:(c + 1) * NLOC]), KC)
        in_maps.append({
            "xdh": np.concatenate(xd_blocks, axis=1),
            "wuh": np.concatenate(wu_blocks, axis=1),
            "wdh": np.concatenate(wd_blocks, axis=1),
            "xsh": xsh,
            "wsuh": wsuh,
            "wsdh": wsdh,
        })

    if caps not in _BUILD_CACHE:
        _BUILD_CACHE[caps] = _build(caps)
    nc = _BUILD_CACHE[caps]

    res = run_bass_kernel_spmd(nc, in_maps, list(range(NCORES)))
    if res.exec_time_ns is not None:
        print(f"HW exec time: {res.exec_time_ns} ns", flush=True)

    # ---- host combine (unshard): gather per-slot rows, weight, sum ----
    y_ts = np.empty((N * TOPK, C), dtype=np.float32)
    for c in range(NCORES):
        yr = res.results[c]["yrh"]
        for j in range(ELOC):
            e = expert_of[c][j]
            n = int(counts[e])
            seg = yr[:, KC * offs[j]:KC * offs[j] + KC * caps[j]]
            seg = seg.reshape(P, KC, caps[j])[:, :, :n]
            y_ts[expert_slots[e]] = (
                seg.transpose(2, 1, 0).reshape(n, C).astype(np.float32))
    routed = (y_ts.reshape(N, TOPK, C)
              * w.reshape(N, TOPK, 1).astype(np.float32)).sum(axis=1)
    shared = np.concatenate(
        [r["ysh"] for r in res.results], axis=0).astype(np.float32)
    return (shared + routed).reshape(B, T, C).astype(np.float32)



# revision 16
# speedup vs baseline: 1.4331x; 1.4331x over previous
"""MoE routing kernel for Trainium2, expert-parallel across 8 NeuronCores.

Sharding: experts are sorted by dispatch count and dealt round-robin so that
slot position j on every core has the same padded capacity caps[j] (baked into
the SPMD program). The gate/top-k/dispatch runs on host as part of sharding;
each core receives its experts' dispatched token rows and expert weights in
fp8-e4m3 (the routed path carries ~28% of the output norm, so e4m3 keeps the
end-to-end rel err ~1.9e-2), plus a slice of tokens for the (replicated,
bf16) shared expert. Expert up/down GEMMs run in DoubleRow fp8 perf mode
(2 contraction tiles per instruction); w_up gate cols are scaled x64 and
v cols x16 so all fp8 operands sit in e4m3 normal range (silu un-scales via
the activation's scale arg, the residual x1024 on the outputs is divided out
in the host combine). Shared-expert passes are interleaved between expert
phases so the tensor engine stays busy while expert DMA streams in. Host
gathers per-slot outputs and does the weighted combine (unshard).
"""

import numpy as np
import ml_dtypes

import bass_rust
import concourse.bass as bass
import concourse.mybir as mybir
from concourse.tile import TileContext
from concourse.vector_clock import ScopedClock
from concourse.bass_utils import run_bass_kernel_spmd

B, T, C = 2, 2048, 2048
N = B * T
E, H, HS = 64, 256, 512
TOPK = 6
NCORES = 8
ELOC = E // NCORES  # 8 experts per core
NLOC = N // NCORES  # 512 tokens per core for the shared expert
BF16 = mybir.dt.bfloat16
FP8 = mybir.dt.float8e4
F32 = mybir.dt.float32
U16 = mybir.dt.uint16
P = 128
KC = C // P  # 16 contraction chunks over C

_BF16_NP = ml_dtypes.bfloat16
_FP8_NP = ml_dtypes.float8_e4m3  # TRN FP8_EXP4 semantics (max 240)

GS = 64.0   # w_up gate-half quantization scale
VS = 16.0   # w_up v-half scale (keeps |h8| = |silu(g)*v*16| well under 240)
DS = 64.0   # w_down scale; yrh carries VS*DS = 1024x values, host divides
YSCALE = VS * DS

DR = mybir.MatmulPerfMode.DoubleRow

# shared-up pair order: pair i computes y chunk m=i and gate chunk m=4+i of
# the 2*HS=1024 up-projection columns
_MPAIR = [0, 4, 1, 5, 2, 6, 3, 7]


# --------------------------------------------------------------------------
# Tile tail-drain fix: this walrus build allows at most one semaphore wait per
# instruction (none on Drain). Tile's end-of-context drain carries the whole
# global clock; emit a chain of single-wait NOPs on SP instead.
# --------------------------------------------------------------------------
def _patched_drain_and_barrier(self, tick_clock, wait_clock):
    carrier = self.nc.sync.nop(nofuse=True, hint="tail_wait_0")
    wait_clock.add_sem_waits(carrier.ins, ScopedClock({None: tick_clock.global_clock}))
    si = carrier.ins.sync_info
    waits = list(si.on_wait) if si else []
    upds = list(si.on_update) if si else []
    carrier.ins.sync_info = bass_rust.SyncInfo(on_wait=waits[:1], on_update=upds)
    for i, w in enumerate(waits[1:]):
        n2 = self.nc.sync.nop(nofuse=True, hint=f"tail_wait_{i + 1}")
        n2.ins.sync_info = bass_rust.SyncInfo(on_wait=[w], on_update=[])

    self.nc.sync.drain()
    self.nc.all_engine_barrier()
    assert self.sems is not None
    popped = self.nc._tile_sem_poison_stack.pop()
    assert popped is self._sem_poison
    # sems still cleared for NEFF re-execution, but the trailing all-engine
    # barrier is redundant: engines are already drained and barriered, and
    # the gpsimd clear halts on its own before the NEFF ends
    self.nc.clear_and_free_semaphores(list(self.sems.allocated().values()))


_orig_add_instruction = TileContext._add_instruction


def _patched_add_instruction(self, inst):
    si = getattr(inst, "sync_info", None)
    if si is not None and len(si.on_wait) > 1:
        waits = list(si.on_wait)
        for w in waits[:-1]:
            nop = mybir.InstNoOp(
                name=self.nc.get_next_instruction_name(), ins=[], outs=[])
            nop.engine = inst.engine
            nop.sync_info = bass_rust.SyncInfo(on_wait=[w], on_update=[])
            _orig_add_instruction(self, nop)
        inst.sync_info = bass_rust.SyncInfo(
            on_wait=[waits[-1]], on_update=list(si.on_update))
    _orig_add_instruction(self, inst)


def _install_drain_fix():
    if getattr(TileContext, "_drain_fix_installed", False):
        return
    TileContext._drain_and_barrier = _patched_drain_and_barrier
    TileContext._add_instruction = _patched_add_instruction
    TileContext._drain_fix_installed = True


# --------------------------------------------------------------------------
# Device kernel
# --------------------------------------------------------------------------
_BUILD_CACHE = {}


def _build(caps):
    """Per-core Bass program; caps[j] = padded capacity of slot position j."""
    _install_drain_fix()
    nc = bass.Bass()

    offs = [0]
    for cp in caps:
        offs.append(offs[-1] + cp)
    S = offs[-1]

    # all HBM tensors are partition-major: [128, X] with large contiguous
    # per-partition runs so DMA packets are 4-16KB
    xdh = nc.declare_dram_parameter("xdh", [P, KC * S], FP8, isOutput=False)
    wuh = nc.declare_dram_parameter("wuh", [P, ELOC * KC * 2 * H], FP8, isOutput=False)
    wdh = nc.declare_dram_parameter("wdh", [P, ELOC * 2 * C], FP8, isOutput=False)
    xsh = nc.declare_dram_parameter("xsh", [P, KC * NLOC], BF16, isOutput=False)
    wsuh = nc.declare_dram_parameter("wsuh", [P, KC * 2 * HS], BF16, isOutput=False)
    wsdh = nc.declare_dram_parameter("wsdh", [P, 4 * C], BF16, isOutput=False)
    yrh = nc.declare_dram_parameter("yrh", [P, KC * S], BF16, isOutput=True)
    ysh = nc.declare_dram_parameter("ysh", [NLOC, C], BF16, isOutput=True)

    with TileContext(nc) as tc:
        with (
            tc.tile_pool(name="xsg_sb", bufs=4) as xsg_pool,
            tc.tile_pool(name="wsug_sb", bufs=2) as wsug_pool,
            tc.tile_pool(name="wsd_sb", bufs=1) as wsd_pool,
            tc.tile_pool(name="hsh_sb", bufs=1) as hsh_pool,
            tc.tile_pool(name="osh_sb", bufs=3) as osh_pool,
            tc.tile_pool(name="xd_sb", bufs=4) as xd_pool,
            tc.tile_pool(name="wu_sb", bufs=4) as wu_pool,
            tc.tile_pool(name="wd_sb", bufs=4) as wd_pool,
            tc.tile_pool(name="yo_sb", bufs=3) as yo_pool,
            tc.tile_pool(name="h_sb", bufs=2) as h_pool,
            tc.tile_pool(name="sg_sb", bufs=2) as sg_pool,
            tc.tile_pool(name="pu", bufs=4, space="PSUM") as pu_pool,
            tc.tile_pool(name="pd", bufs=4, space="PSUM") as pd_pool,
        ):
            xd_t = [None] * ELOC
            wu_t = [None] * ELOC
            wd_t = [None] * ELOC
            h_t = [None] * ELOC

            def load_expert(j, split=1):
                # split>1 chops xd/wu into k-range pieces so up_expert(j)'s
                # first matmuls (gated by subtile deps) start before the whole
                # expert has landed — used for expert 0 to shrink startup
                cap = caps[j]
                xd_t[j] = xd_pool.tile([P, KC, cap], FP8, tag="xd",
                                       name=f"xd_{j}")
                wu_t[j] = wu_pool.tile([P, KC, 512], FP8, tag="wu",
                                       name=f"wu_{j}")
                kq = KC // split
                for q in range(split):
                    nc.sync.dma_start(
                        out=xd_t[j][:, q * kq:(q + 1) * kq, :],
                        in_=xdh[:, KC * offs[j] + q * kq * cap:
                                KC * offs[j] + (q + 1) * kq * cap])
                    nc.sync.dma_start(
                        out=wu_t[j][:, q * kq:(q + 1) * kq, :],
                        in_=wuh[:, j * KC * 512 + q * kq * 512:
                                j * KC * 512 + (q + 1) * kq * 512])
                wd_t[j] = wd_pool.tile([P, 2, C], FP8, tag="wd",
                                       name=f"wd_{j}")
                nc.sync.dma_start(
                    out=wd_t[j][:], in_=wdh[:, j * 2 * C:(j + 1) * 2 * C])

            def up_expert(j):
                # DoubleRow fp8: each matmul contracts 2 k-tiles (256 rows);
                # psum g = GS*g_true, v = VS*v_true; silu un-scales g via
                # activation scale, h8 = silu(g)*v*VS stored fp8
                cap = caps[j]
                xd, wu = xd_t[j], wu_t[j]
                assert cap <= 512
                h8 = h_pool.tile([P, 2, cap], FP8, tag="h", name=f"h_{j}")
                for hf in range(2):
                    p_g = pu_pool.tile([P, cap], F32, space="PSUM",
                                       tag="pu", name=f"pug_{j}_{hf}")
                    p_v = pu_pool.tile([P, cap], F32, space="PSUM",
                                       tag="pu", name=f"puv_{j}_{hf}")
                    for k2 in range(KC // 2):
                        rhs = xd[:, 2 * k2:2 * k2 + 2, :]
                        nc.tensor.matmul(
                            out=p_g[:],
                            lhsT=wu[:, 2 * k2:2 * k2 + 2, hf * P:(hf + 1) * P],
                            rhs=rhs,
                            start=(k2 == 0), stop=(k2 == KC // 2 - 1),
                            perf_mode=DR)
                        nc.tensor.matmul(
                            out=p_v[:],
                            lhsT=wu[:, 2 * k2:2 * k2 + 2,
                                    2 * P + hf * P:2 * P + (hf + 1) * P],
                            rhs=rhs,
                            start=(k2 == 0), stop=(k2 == KC // 2 - 1),
                            perf_mode=DR)
                    sg = sg_pool.tile([P, cap], F32, tag="sg",
                                      name=f"sg_{j}_{hf}")
                    nc.scalar.activation(sg[:], p_g[:],
                                         mybir.ActivationFunctionType.Silu,
                                         scale=1.0 / GS)
                    nc.vector.tensor_mul(h8[:, hf, :], sg[:], p_v[:])
                h_t[j] = h8

            store_pending = []

            def flush_stores(keep=0):
                # scalar-queue issue: the scalar FIFO reaches this only after
                # the preceding down's copies, so the store is time-gated to
                # its program position and its semaphores are already met
                # (no head-of-line blocking); reads keep HBM to themselves
                # until the issue point
                while len(store_pending) > keep:
                    j2, yo2 = store_pending.pop(0)
                    cap2 = caps[j2]
                    b2 = KC * offs[j2]
                    for hh in range(2):
                        lo = hh * 8 * cap2
                        nc.scalar.dma_start(
                            out=yrh[:, b2 + lo:b2 + lo + 8 * cap2],
                            in_=yo2[:, lo:lo + 8 * cap2])

            def down_expert(j, eager_store=False):
                # one DoubleRow matmul per 128-col C chunk (contraction H=256
                # = 2 k-tiles); psum->sbuf copies split vector/scalar
                cap = caps[j]
                base = KC * offs[j]
                wd, h8 = wd_t[j], h_t[j]
                flush_stores(keep=1)
                yo = yo_pool.tile([P, KC * cap], BF16, tag="yo",
                                  name=f"yo_{j}")
                for cc in range(KC):
                    pd = pd_pool.tile([P, 512], F32, space="PSUM",
                                      tag="pd", name=f"pd_{j}_{cc}")
                    nc.tensor.matmul(
                        out=pd[:, 0:cap],
                        lhsT=wd[:, :, cc * P:(cc + 1) * P],
                        rhs=h8[:],
                        start=True, stop=True, perf_mode=DR)
                    # psum f32 reads cost 2 cyc/elem, but bf16 conversion
                    # only needs each f32's high half-word: a strided u16
                    # bitcast copy (truncation, not RNE - error negligible)
                    # halves the read bytes; split 9/7 vector/scalar so
                    # neither FIFO paces the matmuls (gpsimd cannot read PSUM)
                    dst = yo[:, cc * cap:(cc + 1) * cap].bitcast(U16)
                    srcp = pd[:, 0:cap].bitcast(U16)[:, 1::2]
                    if cc % 16 in (1, 4, 6, 9, 11, 12, 14):
                        nc.scalar.copy(dst, srcp)
                    else:
                        nc.vector.tensor_copy(out=dst, in_=srcp)
                # stores go on the (otherwise idle) gpsimd queue, deferred:
                # reads are the DMA critical path until the last expert has
                # landed, so expert j's store issues only alongside expert
                # j+1's down (flush_store below); eager for the tail
                store_pending.append((j, yo))
                if eager_store:
                    flush_stores()

            # ------------- shared expert helpers ----------------------------
            xs_q = [None] * 4
            wsu_p = [None] * 4

            def load_shared_x():
                for qq in range(4):
                    t = xsg_pool.tile([P, 4 * NLOC], BF16, tag="xsg",
                                      name=f"xs_{qq}")
                    nc.sync.dma_start(
                        out=t[:],
                        in_=xsh[:, qq * 4 * NLOC:(qq + 1) * 4 * NLOC])
                    xs_q[qq] = t

            def load_shared_pair(pr):
                t = wsug_pool.tile([P, KC * 256], BF16, tag="wsug",
                                   name=f"wsu_{pr}")
                nc.sync.dma_start(
                    out=t[:], in_=wsuh[:, pr * 4096:(pr + 1) * 4096])
                wsu_p[pr] = t

            def su_pass(pr):
                # shared up pair pr: y chunk and gate chunk, 16 k-tiles bf16;
                # grouped (all y then all g) rather than alternating
                ps_y = pu_pool.tile([P, NLOC], F32, space="PSUM", tag="pu",
                                    name=f"ps_y{pr}")
                ps_g = pu_pool.tile([P, NLOC], F32, space="PSUM", tag="pu",
                                    name=f"ps_g{pr}")
                wt = wsu_p[pr]
                for half, ps in ((0, ps_y), (1, ps_g)):
                    for k in range(KC):
                        lo = k * 256 + half * P
                        rhs = xs_q[k // 4][:, (k % 4) * NLOC:(k % 4 + 1) * NLOC]
                        nc.tensor.matmul(
                            out=ps[:], lhsT=wt[:, lo:lo + P], rhs=rhs,
                            start=(k == 0), stop=(k == KC - 1))
                sg = sg_pool.tile([P, NLOC], F32, tag="sg", name=f"sgs_{pr}")
                nc.scalar.activation(sg[:], ps_g[:],
                                     mybir.ActivationFunctionType.Silu)
                nc.vector.tensor_mul(
                    hsh[:, pr * NLOC:(pr + 1) * NLOC], sg[:], ps_y[:])

            def sd_pass(mt):
                # shared down token tile mt: 4 kh contraction chunks
                osh = osh_pool.tile([P, C], BF16, tag="osh",
                                    name=f"osh_{mt}")
                for ncc in range(4):
                    pd = pd_pool.tile([P, 512], F32, space="PSUM",
                                      tag="pd", name=f"pds_{mt}_{ncc}")
                    for kh in range(4):
                        nc.tensor.matmul(
                            out=pd[:],
                            lhsT=hsh[:, kh * NLOC + mt * P:
                                     kh * NLOC + (mt + 1) * P],
                            rhs=wsdh_t[:, kh * C + ncc * 512:
                                       kh * C + (ncc + 1) * 512],
                            start=(kh == 0), stop=(kh == 3))
                    dst = osh[:, ncc * 512:(ncc + 1) * 512].bitcast(U16)
                    srcp = pd[:].bitcast(U16)[:, 1::2]
                    if ncc % 2 == 1:
                        nc.scalar.copy(dst, srcp)
                    else:
                        nc.vector.tensor_copy(out=dst, in_=srcp)
                nc.gpsimd.dma_start(
                    out=ysh[mt * P:(mt + 1) * P, :], in_=osh[:])

            # ------------- load issue order (single sync FIFO queue) --------
            # expert 0 first (split so its first k-chunks land early), then
            # expert 1; shared tiles woven in just ahead of consumption;
            # later experts gated by pool slot release line up behind
            load_expert(0, split=4)
            load_expert(1)
            load_shared_x()
            load_shared_pair(0)
            load_expert(2)
            load_shared_pair(1)
            load_expert(3)
            load_shared_pair(2)
            load_expert(4)
            load_shared_pair(3)
            load_expert(5)
            wsdh_t = wsd_pool.tile([P, 4 * C], BF16, tag="wsd")
            nc.sync.dma_start(out=wsdh_t[:], in_=wsdh[:])
            load_expert(6)
            load_expert(7)

            hsh = hsh_pool.tile([P, 4 * NLOC], BF16, tag="hsh")

            # ------------- PE warm-up: dummy matmuls with no DMA deps -------
            # cover the preamble + expert-0 load window so the HAM clock gate
            # releases before real work starts
            nc.vector.memset(hsh[:, 0:512], 0.0)
            pwarm = pd_pool.tile([P, 512], F32, space="PSUM", tag="pd",
                                 name="pwarm")
            for wi in range(8):
                nc.tensor.matmul(out=pwarm[:], lhsT=hsh[:, 0:P],
                                 rhs=hsh[:, 0:512], start=True, stop=True)

            # ------------- compute: experts pipelined, shared interleaved ---
            # up_{j+1} issues before down_j so down's psum copies drain
            # behind matmuls; shared-up passes slot in between the early
            # experts (their weights stream while experts compute) and
            # shared-down token tiles between the late ones, so the final
            # output stores are spread instead of clustered in a tail
            up_expert(0)
            up_expert(1)
            down_expert(0)
            su_pass(0)
            up_expert(2)
            down_expert(1)
            su_pass(1)
            up_expert(3)
            down_expert(2)
            su_pass(2)
            up_expert(4)
            down_expert(3)
            su_pass(3)
            up_expert(5)
            down_expert(4)
            sd_pass(0)
            up_expert(6)
            down_expert(5)
            sd_pass(1)
            up_expert(7)
            down_expert(6)
            sd_pass(2)
            down_expert(7, eager_store=True)
            sd_pass(3)
    return nc


# --------------------------------------------------------------------------
# Host wrapper
# --------------------------------------------------------------------------
def _pm(a, nchunk):
    """[nchunk*128, X] row-major -> partition-major [128, nchunk*X]."""
    x = a.shape[1]
    return np.ascontiguousarray(
        a.reshape(nchunk, P, x).transpose(1, 0, 2)).reshape(P, nchunk * x)


def kernel(x, w_gate, w_shared_up, w_shared_down, w_up, w_down):
    x_flat = np.asarray(x, dtype=np.float32).reshape(-1, C)

    # ---- gate: sigmoid scores, top-6, normalized weights (f64 for a stable
    # ordering; ties in the fp32 reference are measure-zero) ----
    logits = x_flat.astype(np.float64) @ np.asarray(w_gate, np.float64)
    scores = 1.0 / (1.0 + np.exp(-logits))
    topk_idx = np.argsort(-scores, axis=-1, kind="stable")[:, :TOPK]
    w = np.take_along_axis(scores, topk_idx, axis=-1)
    w = w / w.sum(-1, keepdims=True)

    # ---- dispatch positions (stable within each expert, slot-major order) --
    flat_e = topk_idx.reshape(-1)
    order = np.argsort(flat_e, kind="stable")
    sorted_e = flat_e[order]
    group_start = np.searchsorted(sorted_e, np.arange(E))
    counts = np.bincount(flat_e, minlength=E)

    token_of_slot = np.arange(N * TOPK) // TOPK
    expert_slots = []   # flat (token,k) slot ids, dispatch order, per expert
    expert_tokens = []
    for e in range(E):
        slots = order[group_start[e]: group_start[e] + counts[e]]
        expert_slots.append(slots)
        expert_tokens.append(token_of_slot[slots])

    # ---- balanced expert->core assignment: sort by count desc, deal 8 at a
    # time; slot position j has the same padded cap on every core.
    # caps multiple of 16 (DoubleRow AP stride) and >=128 (FWL crossover) ----
    ranks = np.argsort(-counts, kind="stable")
    expert_of = [[int(ranks[8 * j + c]) for j in range(ELOC)]
                 for c in range(NCORES)]
    caps = tuple(
        max(128, int(-(-int(counts[ranks[8 * j]]) // 16) * 16))
        for j in range(ELOC))
    offs = [0]
    for cp in caps:
        offs.append(offs[-1] + cp)

    # ---- build per-core inputs ----
    xT8 = np.ascontiguousarray(x_flat.T).astype(_FP8_NP)     # [C, N] fp8
    xT_bf = np.ascontiguousarray(x_flat.T).astype(_BF16_NP)  # [C, N] bf16
    wsu_f = np.asarray(w_shared_up, np.float32).astype(_BF16_NP)
    wsd_f = np.asarray(w_shared_down, np.float32).astype(_BF16_NP)

    # shared-up weights in pair/k-major order (see _MPAIR)
    wr = wsu_f.reshape(KC, P, 8, P)[:, :, _MPAIR, :]        # [k, p, 8, 128]
    wsuh = np.ascontiguousarray(
        wr.reshape(KC, P, 4, 2 * P).transpose(1, 2, 0, 3)).reshape(P, KC * 2 * HS)
    wsdh = _pm(wsd_f, 4)

    # fp8 expert weights: gate cols xGS, v cols xVS; w_down xDS
    upscale = np.concatenate([np.full(H, GS, np.float32),
                              np.full(H, VS, np.float32)])

    in_maps = []
    for c in range(NCORES):
        xd_blocks = []
        wu_blocks = []
        wd_blocks = []
        for j in range(ELOC):
            e = expert_of[c][j]
            tok = expert_tokens[e]
            n = len(tok)
            blk = np.zeros((P, KC, caps[j]), dtype=_FP8_NP)
            blk[:, :, :n] = xT8[:, tok].reshape(KC, P, n).transpose(1, 0, 2)
            xd_blocks.append(blk.reshape(P, -1))
            wu_blocks.append(
                _pm(np.asarray(w_up[e], np.float32) * upscale, KC)
                .astype(_FP8_NP))
            wd_blocks.append(
                _pm(np.asarray(w_down[e], np.float32) * DS, 2)
                .astype(_FP8_NP))
        xsh = _pm(np.ascontiguousarray(
            xT_bf[:, c * NLOC:(c + 1) * NLOC]), KC)
        in_maps.append({
            "xdh": np.concatenate(xd_blocks, axis=1),
            "wuh": np.concatenate(wu_blocks, axis=1),
            "wdh": np.concatenate(wd_blocks, axis=1),
            "xsh": xsh,
            "wsuh": wsuh,
            "wsdh": wsdh,
        })

    if caps not in _BUILD_CACHE:
        _BUILD_CACHE[caps] = _build(caps)
    nc = _BUILD_CACHE[caps]

    res = run_bass_kernel_spmd(nc, in_maps, list(range(NCORES)))
    if res.exec_time_ns is not None:
        print(f"HW exec time: {res.exec_time_ns} ns", flush=True)

    # ---- host combine (unshard): gather per-slot rows, un-scale, weight, sum
    y_ts = np.empty((N * TOPK, C), dtype=np.float32)
    for c in range(NCORES):
        yr = res.results[c]["yrh"]
        for j in range(ELOC):
            e = expert_of[c][j]
            n = int(counts[e])
            seg = yr[:, KC * offs[j]:KC * offs[j] + KC * caps[j]]
            seg = seg.reshape(P, KC, caps[j])[:, :, :n]
            y_ts[expert_slots[e]] = (
                seg.transpose(2, 1, 0).reshape(n, C).astype(np.float32))
    wq = (w / YSCALE).astype(np.float32)
    routed = (y_ts.reshape(N, TOPK, C) * wq.reshape(N, TOPK, 1)).sum(axis=1)
    shared = np.concatenate(
        [r["ysh"] for r in res.results], axis=0).astype(np.float32)
    return (shared + routed).reshape(B, T, C).astype(np.float32)


# revision 17
# speedup vs baseline: 1.4622x; 1.0203x over previous
"""MoE routing kernel for Trainium2, expert-parallel across 8 NeuronCores.

Sharding: experts are sorted by dispatch count and dealt round-robin so that
slot position j on every core has the same padded capacity caps[j] (baked into
the SPMD program). The gate/top-k/dispatch runs on host as part of sharding;
each core receives its experts' dispatched token rows and expert weights in
fp8-e4m3 (the routed path carries ~28% of the output norm, so e4m3 keeps the
end-to-end rel err ~1.9e-2), plus a slice of tokens for the (replicated,
bf16) shared expert. Expert up/down GEMMs run in DoubleRow fp8 perf mode
(2 contraction tiles per instruction); w_up gate cols are scaled x64 and
v cols x16 so all fp8 operands sit in e4m3 normal range (silu un-scales via
the activation's scale arg, the residual x1024 on the outputs is divided out
in the host combine). Shared-expert passes are interleaved between expert
phases so the tensor engine stays busy while expert DMA streams in. Host
gathers per-slot outputs and does the weighted combine (unshard).
"""

import numpy as np
import ml_dtypes

import bass_rust
import concourse.bass as bass
import concourse.mybir as mybir
from concourse.tile import TileContext
from concourse.vector_clock import ScopedClock
from concourse.bass_utils import run_bass_kernel_spmd

B, T, C = 2, 2048, 2048
N = B * T
E, H, HS = 64, 256, 512
TOPK = 6
NCORES = 8
ELOC = E // NCORES  # 8 experts per core
NLOC = N // NCORES  # 512 tokens per core for the shared expert
BF16 = mybir.dt.bfloat16
FP8 = mybir.dt.float8e4
F32 = mybir.dt.float32
U16 = mybir.dt.uint16
P = 128
KC = C // P  # 16 contraction chunks over C

_BF16_NP = ml_dtypes.bfloat16
_FP8_NP = ml_dtypes.float8_e4m3  # TRN FP8_EXP4 semantics (max 240)

GS = 64.0   # w_up gate-half quantization scale
VS = 16.0   # w_up v-half scale (keeps |h8| = |silu(g)*v*16| well under 240)
DS = 64.0   # w_down scale; yrh carries VS*DS = 1024x values, host divides
YSCALE = VS * DS

DR = mybir.MatmulPerfMode.DoubleRow

# shared-up pair order: pair i computes y chunk m=i and gate chunk m=4+i of
# the 2*HS=1024 up-projection columns
_MPAIR = [0, 4, 1, 5, 2, 6, 3, 7]


# --------------------------------------------------------------------------
# Tile tail-drain fix: this walrus build allows at most one semaphore wait per
# instruction (none on Drain). Tile's end-of-context drain carries the whole
# global clock; emit a chain of single-wait NOPs on SP instead.
# --------------------------------------------------------------------------
def _patched_drain_and_barrier(self, tick_clock, wait_clock):
    carrier = self.nc.sync.nop(nofuse=True, hint="tail_wait_0")
    wait_clock.add_sem_waits(carrier.ins, ScopedClock({None: tick_clock.global_clock}))
    si = carrier.ins.sync_info
    waits = list(si.on_wait) if si else []
    upds = list(si.on_update) if si else []
    carrier.ins.sync_info = bass_rust.SyncInfo(on_wait=waits[:1], on_update=upds)
    for i, w in enumerate(waits[1:]):
        n2 = self.nc.sync.nop(nofuse=True, hint=f"tail_wait_{i + 1}")
        n2.ins.sync_info = bass_rust.SyncInfo(on_wait=[w], on_update=[])

    self.nc.sync.drain()
    self.nc.all_engine_barrier()
    assert self.sems is not None
    popped = self.nc._tile_sem_poison_stack.pop()
    assert popped is self._sem_poison
    # sems still cleared for NEFF re-execution, but the trailing all-engine
    # barrier is redundant: engines are already drained and barriered, and
    # the gpsimd clear halts on its own before the NEFF ends
    self.nc.clear_and_free_semaphores(list(self.sems.allocated().values()))


_orig_add_instruction = TileContext._add_instruction


def _patched_add_instruction(self, inst):
    si = getattr(inst, "sync_info", None)
    if si is not None and len(si.on_wait) > 1:
        waits = list(si.on_wait)
        for w in waits[:-1]:
            nop = mybir.InstNoOp(
                name=self.nc.get_next_instruction_name(), ins=[], outs=[])
            nop.engine = inst.engine
            nop.sync_info = bass_rust.SyncInfo(on_wait=[w], on_update=[])
            _orig_add_instruction(self, nop)
        inst.sync_info = bass_rust.SyncInfo(
            on_wait=[waits[-1]], on_update=list(si.on_update))
    _orig_add_instruction(self, inst)


def _install_drain_fix():
    if getattr(TileContext, "_drain_fix_installed", False):
        return
    TileContext._drain_and_barrier = _patched_drain_and_barrier
    TileContext._add_instruction = _patched_add_instruction
    TileContext._drain_fix_installed = True


# --------------------------------------------------------------------------
# Device kernel
# --------------------------------------------------------------------------
_BUILD_CACHE = {}


def _build(caps):
    """Per-core Bass program; caps[j] = padded capacity of slot position j."""
    _install_drain_fix()
    nc = bass.Bass()

    offs = [0]
    for cp in caps:
        offs.append(offs[-1] + cp)
    S = offs[-1]

    # all HBM tensors are partition-major: [128, X] with large contiguous
    # per-partition runs so DMA packets are 4-16KB
    xdh = nc.declare_dram_parameter("xdh", [P, KC * S], FP8, isOutput=False)
    wuh = nc.declare_dram_parameter("wuh", [P, ELOC * KC * 2 * H], FP8, isOutput=False)
    wdh = nc.declare_dram_parameter("wdh", [P, ELOC * 2 * C], FP8, isOutput=False)
    xsh = nc.declare_dram_parameter("xsh", [P, KC * NLOC], BF16, isOutput=False)
    wsuh = nc.declare_dram_parameter("wsuh", [P, KC * 2 * HS], BF16, isOutput=False)
    wsdh = nc.declare_dram_parameter("wsdh", [P, 4 * C], BF16, isOutput=False)
    yrh = nc.declare_dram_parameter("yrh", [P, KC * S], BF16, isOutput=True)
    ysh = nc.declare_dram_parameter("ysh", [NLOC, C], BF16, isOutput=True)

    with TileContext(nc) as tc:
        with (
            tc.tile_pool(name="xsg_sb", bufs=4) as xsg_pool,
            tc.tile_pool(name="wsug_sb", bufs=2) as wsug_pool,
            tc.tile_pool(name="wsd_sb", bufs=1) as wsd_pool,
            tc.tile_pool(name="hsh_sb", bufs=1) as hsh_pool,
            tc.tile_pool(name="osh_sb", bufs=3) as osh_pool,
            tc.tile_pool(name="xd_sb", bufs=4) as xd_pool,
            tc.tile_pool(name="wu_sb", bufs=4) as wu_pool,
            tc.tile_pool(name="wd_sb", bufs=4) as wd_pool,
            tc.tile_pool(name="yo_sb", bufs=3) as yo_pool,
            tc.tile_pool(name="h_sb", bufs=2) as h_pool,
            tc.tile_pool(name="sg_sb", bufs=2) as sg_pool,
            tc.tile_pool(name="pu", bufs=4, space="PSUM") as pu_pool,
            tc.tile_pool(name="pd", bufs=4, space="PSUM") as pd_pool,
        ):
            xd_t = [None] * ELOC
            wu_t = [None] * ELOC
            wd_t = [None] * ELOC
            h_t = [None] * ELOC

            def load_expert(j, split=1):
                # split>1 chops xd/wu into k-range pieces so up_expert(j)'s
                # first matmuls (gated by subtile deps) start before the whole
                # expert has landed — used for expert 0 to shrink startup
                cap = caps[j]
                xd_t[j] = xd_pool.tile([P, KC, cap], FP8, tag="xd",
                                       name=f"xd_{j}")
                wu_t[j] = wu_pool.tile([P, KC, 512], FP8, tag="wu",
                                       name=f"wu_{j}")
                kq = KC // split
                for q in range(split):
                    nc.sync.dma_start(
                        out=xd_t[j][:, q * kq:(q + 1) * kq, :],
                        in_=xdh[:, KC * offs[j] + q * kq * cap:
                                KC * offs[j] + (q + 1) * kq * cap])
                    nc.sync.dma_start(
                        out=wu_t[j][:, q * kq:(q + 1) * kq, :],
                        in_=wuh[:, j * KC * 512 + q * kq * 512:
                                j * KC * 512 + (q + 1) * kq * 512])
                wd_t[j] = wd_pool.tile([P, 2, C], FP8, tag="wd",
                                       name=f"wd_{j}")
                nc.sync.dma_start(
                    out=wd_t[j][:], in_=wdh[:, j * 2 * C:(j + 1) * 2 * C])

            def up_expert(j):
                # DoubleRow fp8: each matmul contracts 2 k-tiles (256 rows);
                # psum g = GS*g_true, v = VS*v_true; silu un-scales g via
                # activation scale, h8 = silu(g)*v*VS stored fp8
                cap = caps[j]
                xd, wu = xd_t[j], wu_t[j]
                assert cap <= 512
                h8 = h_pool.tile([P, 2, cap], FP8, tag="h", name=f"h_{j}")
                for hf in range(2):
                    p_g = pu_pool.tile([P, cap], F32, space="PSUM",
                                       tag="pu", name=f"pug_{j}_{hf}")
                    p_v = pu_pool.tile([P, cap], F32, space="PSUM",
                                       tag="pu", name=f"puv_{j}_{hf}")
                    for k2 in range(KC // 2):
                        rhs = xd[:, 2 * k2:2 * k2 + 2, :]
                        nc.tensor.matmul(
                            out=p_g[:],
                            lhsT=wu[:, 2 * k2:2 * k2 + 2, hf * P:(hf + 1) * P],
                            rhs=rhs,
                            start=(k2 == 0), stop=(k2 == KC // 2 - 1),
                            perf_mode=DR)
                        nc.tensor.matmul(
                            out=p_v[:],
                            lhsT=wu[:, 2 * k2:2 * k2 + 2,
                                    2 * P + hf * P:2 * P + (hf + 1) * P],
                            rhs=rhs,
                            start=(k2 == 0), stop=(k2 == KC // 2 - 1),
                            perf_mode=DR)
                    sg = sg_pool.tile([P, cap], F32, tag="sg",
                                      name=f"sg_{j}_{hf}")
                    nc.scalar.activation(sg[:], p_g[:],
                                         mybir.ActivationFunctionType.Silu,
                                         scale=1.0 / GS)
                    nc.vector.tensor_mul(h8[:, hf, :], sg[:], p_v[:])
                h_t[j] = h8

            store_pending = []

            def flush_stores(keep=0, gate=None):
                # gpsimd-queue issue: gpsimd is otherwise idle, so a dummy
                # gate copy (reads the current expert's h8, ready ~now) holds
                # the queue until this point in time; the store semaphores
                # are then already met and reads keep HBM to themselves
                # until the issue point
                if gate is not None and len(store_pending) > keep:
                    nc.gpsimd.tensor_copy(out=gate_t[:], in_=gate)
                while len(store_pending) > keep:
                    j2, yo2 = store_pending.pop(0)
                    cap2 = caps[j2]
                    b2 = KC * offs[j2]
                    for hh in range(2):
                        lo = hh * 8 * cap2
                        nc.gpsimd.dma_start(
                            out=yrh[:, b2 + lo:b2 + lo + 8 * cap2],
                            in_=yo2[:, lo:lo + 8 * cap2])

            def down_expert(j, eager_store=False):
                # one DoubleRow matmul per 128-col C chunk (contraction H=256
                # = 2 k-tiles); psum->sbuf copies split vector/scalar
                cap = caps[j]
                base = KC * offs[j]
                wd, h8 = wd_t[j], h_t[j]
                flush_stores(keep=1, gate=h8[:, 0, 0:64])
                yo = yo_pool.tile([P, KC * cap], BF16, tag="yo",
                                  name=f"yo_{j}")
                for cc in range(KC):
                    pd = pd_pool.tile([P, 512], F32, space="PSUM",
                                      tag="pd", name=f"pd_{j}_{cc}")
                    nc.tensor.matmul(
                        out=pd[:, 0:cap],
                        lhsT=wd[:, :, cc * P:(cc + 1) * P],
                        rhs=h8[:],
                        start=True, stop=True, perf_mode=DR)
                    # copies pace the down phase (psum reads ~1.2ns/elem
                    # on both engines): split 8/8 vector/scalar and let them
                    # drain into the next up phase via the pd buf slack
                    # (gpsimd cannot read PSUM)
                    dst = yo[:, cc * cap:(cc + 1) * cap]
                    if cc % 2 == 1:
                        nc.scalar.copy(dst, pd[:, 0:cap])
                    else:
                        nc.vector.tensor_copy(out=dst, in_=pd[:, 0:cap])
                # stores go on the (otherwise idle) gpsimd queue, deferred:
                # reads are the DMA critical path until the last expert has
                # landed, so expert j's store issues only alongside expert
                # j+1's down (flush_store below); eager for the tail
                store_pending.append((j, yo))
                if eager_store:
                    flush_stores()

            # ------------- shared expert helpers ----------------------------
            xs_q = [None] * 4
            wsu_p = [None] * 4

            def load_shared_x():
                for qq in range(4):
                    t = xsg_pool.tile([P, 4 * NLOC], BF16, tag="xsg",
                                      name=f"xs_{qq}")
                    nc.sync.dma_start(
                        out=t[:],
                        in_=xsh[:, qq * 4 * NLOC:(qq + 1) * 4 * NLOC])
                    xs_q[qq] = t

            def load_shared_pair(pr):
                t = wsug_pool.tile([P, KC * 256], BF16, tag="wsug",
                                   name=f"wsu_{pr}")
                nc.sync.dma_start(
                    out=t[:], in_=wsuh[:, pr * 4096:(pr + 1) * 4096])
                wsu_p[pr] = t

            def su_pass(pr):
                # shared up pair pr: y chunk and gate chunk, 16 k-tiles bf16;
                # grouped (all y then all g) rather than alternating
                ps_y = pu_pool.tile([P, NLOC], F32, space="PSUM", tag="pu",
                                    name=f"ps_y{pr}")
                ps_g = pu_pool.tile([P, NLOC], F32, space="PSUM", tag="pu",
                                    name=f"ps_g{pr}")
                wt = wsu_p[pr]
                for half, ps in ((0, ps_y), (1, ps_g)):
                    for k in range(KC):
                        lo = k * 256 + half * P
                        rhs = xs_q[k // 4][:, (k % 4) * NLOC:(k % 4 + 1) * NLOC]
                        nc.tensor.matmul(
                            out=ps[:], lhsT=wt[:, lo:lo + P], rhs=rhs,
                            start=(k == 0), stop=(k == KC - 1))
                sg = sg_pool.tile([P, NLOC], F32, tag="sg", name=f"sgs_{pr}")
                nc.scalar.activation(sg[:], ps_g[:],
                                     mybir.ActivationFunctionType.Silu)
                nc.vector.tensor_mul(
                    hsh[:, pr * NLOC:(pr + 1) * NLOC], sg[:], ps_y[:])

            def sd_pass(mt):
                # shared down token tile mt: 4 kh contraction chunks
                osh = osh_pool.tile([P, C], BF16, tag="osh",
                                    name=f"osh_{mt}")
                for ncc in range(4):
                    pd = pu_pool.tile([P, 512], F32, space="PSUM",
                                      tag="pu", name=f"pds_{mt}_{ncc}")
                    for kh in range(4):
                        nc.tensor.matmul(
                            out=pd[:],
                            lhsT=hsh[:, kh * NLOC + mt * P:
                                     kh * NLOC + (mt + 1) * P],
                            rhs=wsdh_t[:, kh * C + ncc * 512:
                                       kh * C + (ncc + 1) * 512],
                            start=(kh == 0), stop=(kh == 3))
                    dst = osh[:, ncc * 512:(ncc + 1) * 512]
                    if ncc % 2 == 1:
                        nc.scalar.copy(dst, pd[:])
                    else:
                        nc.vector.tensor_copy(out=dst, in_=pd[:])
                nc.gpsimd.dma_start(
                    out=ysh[mt * P:(mt + 1) * P, :], in_=osh[:])

            # ------------- load issue order (single sync FIFO queue) --------
            # expert 0 first (split so its first k-chunks land early), then
            # expert 1; shared tiles woven in just ahead of consumption;
            # later experts gated by pool slot release line up behind
            load_expert(0, split=4)
            load_expert(1)
            load_shared_pair(0)
            load_shared_x()
            load_expert(2)
            load_shared_pair(1)
            load_expert(3)
            load_shared_pair(2)
            load_expert(4)
            load_shared_pair(3)
            load_expert(5)
            wsdh_t = wsd_pool.tile([P, 4 * C], BF16, tag="wsd")
            nc.sync.dma_start(out=wsdh_t[:], in_=wsdh[:])
            load_expert(6)
            load_expert(7)

            hsh = hsh_pool.tile([P, 4 * NLOC], BF16, tag="hsh")
            gate_t = hsh_pool.tile([P, 64], FP8, tag="gate")

            # ------------- PE warm-up: dummy matmuls with no DMA deps -------
            # cover the preamble + expert-0 load window so the HAM clock gate
            # releases before real work starts
            nc.vector.memset(hsh[:, 0:512], 0.0)
            pwarm = pd_pool.tile([P, 512], F32, space="PSUM", tag="pd",
                                 name="pwarm")
            for wi in range(8):
                nc.tensor.matmul(out=pwarm[:], lhsT=hsh[:, 0:P],
                                 rhs=hsh[:, 0:512], start=True, stop=True)

            # ------------- compute: experts pipelined, shared interleaved ---
            # up_{j+1} issues before down_j so down's psum copies drain
            # behind matmuls; shared-up passes slot in between the early
            # experts (their weights stream while experts compute) and
            # shared-down token tiles between the late ones, so the final
            # output stores are spread instead of clustered in a tail
            up_expert(0)
            up_expert(1)
            down_expert(0)
            su_pass(0)
            up_expert(2)
            down_expert(1)
            su_pass(1)
            up_expert(3)
            down_expert(2)
            su_pass(2)
            up_expert(4)
            down_expert(3)
            su_pass(3)
            up_expert(5)
            down_expert(4)
            sd_pass(0)
            up_expert(6)
            down_expert(5)
            sd_pass(1)
            up_expert(7)
            down_expert(6)
            sd_pass(2)
            down_expert(7, eager_store=True)
            sd_pass(3)
    return nc


# --------------------------------------------------------------------------
# Host wrapper
# --------------------------------------------------------------------------
def _pm(a, nchunk):
    """[nchunk*128, X] row-major -> partition-major [128, nchunk*X]."""
    x = a.shape[1]
    return np.ascontiguousarray(
        a.reshape(nchunk, P, x).transpose(1, 0, 2)).reshape(P, nchunk * x)


def kernel(x, w_gate, w_shared_up, w_shared_down, w_up, w_down):
    x_flat = np.asarray(x, dtype=np.float32).reshape(-1, C)

    # ---- gate: sigmoid scores, top-6, normalized weights (f64 for a stable
    # ordering; ties in the fp32 reference are measure-zero) ----
    logits = x_flat.astype(np.float64) @ np.asarray(w_gate, np.float64)
    scores = 1.0 / (1.0 + np.exp(-logits))
    topk_idx = np.argsort(-scores, axis=-1, kind="stable")[:, :TOPK]
    w = np.take_along_axis(scores, topk_idx, axis=-1)
    w = w / w.sum(-1, keepdims=True)

    # ---- dispatch positions (stable within each expert, slot-major order) --
    flat_e = topk_idx.reshape(-1)
    order = np.argsort(flat_e, kind="stable")
    sorted_e = flat_e[order]
    group_start = np.searchsorted(sorted_e, np.arange(E))
    counts = np.bincount(flat_e, minlength=E)

    token_of_slot = np.arange(N * TOPK) // TOPK
    expert_slots = []   # flat (token,k) slot ids, dispatch order, per expert
    expert_tokens = []
    for e in range(E):
        slots = order[group_start[e]: group_start[e] + counts[e]]
        expert_slots.append(slots)
        expert_tokens.append(token_of_slot[slots])

    # ---- balanced expert->core assignment: sort by count desc, deal 8 at a
    # time; slot position j has the same padded cap on every core.
    # caps multiple of 16 (DoubleRow AP stride) and >=128 (FWL crossover) ----
    ranks = np.argsort(-counts, kind="stable")
    expert_of = [[int(ranks[8 * j + c]) for j in range(ELOC)]
                 for c in range(NCORES)]
    caps = tuple(
        max(128, int(-(-int(counts[ranks[8 * j]]) // 16) * 16))
        for j in range(ELOC))
    offs = [0]
    for cp in caps:
        offs.append(offs[-1] + cp)

    # ---- build per-core inputs ----
    xT8 = np.ascontiguousarray(x_flat.T).astype(_FP8_NP)     # [C, N] fp8
    xT_bf = np.ascontiguousarray(x_flat.T).astype(_BF16_NP)  # [C, N] bf16
    wsu_f = np.asarray(w_shared_up, np.float32).astype(_BF16_NP)
    wsd_f = np.asarray(w_shared_down, np.float32).astype(_BF16_NP)

    # shared-up weights in pair/k-major order (see _MPAIR)
    wr = wsu_f.reshape(KC, P, 8, P)[:, :, _MPAIR, :]        # [k, p, 8, 128]
    wsuh = np.ascontiguousarray(
        wr.reshape(KC, P, 4, 2 * P).transpose(1, 2, 0, 3)).reshape(P, KC * 2 * HS)
    wsdh = _pm(wsd_f, 4)

    # fp8 expert weights: gate cols xGS, v cols xVS; w_down xDS
    upscale = np.concatenate([np.full(H, GS, np.float32),
                              np.full(H, VS, np.float32)])

    in_maps = []
    for c in range(NCORES):
        xd_blocks = []
        wu_blocks = []
        wd_blocks = []
        for j in range(ELOC):
            e = expert_of[c][j]
            tok = expert_tokens[e]
            n = len(tok)
            blk = np.zeros((P, KC, caps[j]), dtype=_FP8_NP)
            blk[:, :, :n] = xT8[:, tok].reshape(KC, P, n).transpose(1, 0, 2)
            xd_blocks.append(blk.reshape(P, -1))
            wu_blocks.append(
                _pm(np.asarray(w_up[e], np.float32) * upscale, KC)
                .astype(_FP8_NP))
            wd_blocks.append(
                _pm(np.asarray(w_down[e], np.float32) * DS, 2)
                .astype(_FP8_NP))
        xsh = _pm(np.ascontiguousarray(
            xT_bf[:, c * NLOC:(c + 1) * NLOC]), KC)
        in_maps.append({
            "xdh": np.concatenate(xd_blocks, axis=1),
            "wuh": np.concatenate(wu_blocks, axis=1),
            "wdh": np.concatenate(wd_blocks, axis=1),
            "xsh": xsh,
            "wsuh": wsuh,
            "wsdh": wsdh,
        })

    if caps not in _BUILD_CACHE:
        _BUILD_CACHE[caps] = _build(caps)
    nc = _BUILD_CACHE[caps]

    res = run_bass_kernel_spmd(nc, in_maps, list(range(NCORES)))
    if res.exec_time_ns is not None:
        print(f"HW exec time: {res.exec_time_ns} ns", flush=True)

    # ---- host combine (unshard): gather per-slot rows, un-scale, weight, sum
    y_ts = np.empty((N * TOPK, C), dtype=np.float32)
    for c in range(NCORES):
        yr = res.results[c]["yrh"]
        for j in range(ELOC):
            e = expert_of[c][j]
            n = int(counts[e])
            seg = yr[:, KC * offs[j]:KC * offs[j] + KC * caps[j]]
            seg = seg.reshape(P, KC, caps[j])[:, :, :n]
            y_ts[expert_slots[e]] = (
                seg.transpose(2, 1, 0).reshape(n, C).astype(np.float32))
    wq = (w / YSCALE).astype(np.float32)
    routed = (y_ts.reshape(N, TOPK, C) * wq.reshape(N, TOPK, 1)).sum(axis=1)
    shared = np.concatenate(
        [r["ysh"] for r in res.results], axis=0).astype(np.float32)
    return (shared + routed).reshape(B, T, C).astype(np.float32)


# revision 21
# speedup vs baseline: 1.5452x; 1.0568x over previous
"""MoE routing kernel for Trainium2, expert-parallel across 8 NeuronCores.

Sharding: experts are sorted by dispatch count and dealt round-robin so that
slot position j on every core has the same padded capacity caps[j] (multiple
of 16 for the DoubleRow AP stride, baked into the SPMD program). The
gate/top-k/dispatch runs on host as part of sharding; each core receives its
experts' dispatched token rows and expert weights in fp8-e4m3 (the routed
path carries only ~28% of the output norm, so e4m3 on both routed GEMMs
keeps end-to-end rel err at ~1.87e-2 < 2e-2), plus a slice of tokens for the
(replicated, bf16-exact) shared expert. Expert up/down GEMMs run in
DoubleRow fp8 perf mode (2 contraction k-tiles per instruction, ~1.9x bf16
matmul rate measured); w_up gate cols are scaled x64 and v cols x16 so all
fp8 operands sit in e4m3 normal range (silu un-scales g via the activation
scale arg; the residual x1024 on the outputs is divided out of the combine
weights on host). Schedule: expert 0's load is split so compute starts as
soon as the first k-chunks land (behind a short PE warm-up that rides out
the preamble + HAM clock ramp); shared-up passes interleave between early
expert phases and shared-down token tiles between late ones, so the tensor
engine stays busy while expert DMA streams and output stores spread out.
The psum->sbuf copies (the down-phase pacer at ~1.2ns/elem) alternate
vector/scalar; shared-down borrows the up-psum pool so it never waits on
down-copy backlog. Host gathers per-slot outputs and does the weighted
combine (unshard).
"""

import numpy as np
import ml_dtypes

import bass_rust
import concourse.bass as bass
import concourse.mybir as mybir
from concourse.tile import TileContext
from concourse.vector_clock import ScopedClock
from concourse.bass_utils import run_bass_kernel_spmd

B, T, C = 2, 2048, 2048
N = B * T
E, H, HS = 64, 256, 512
TOPK = 6
NCORES = 8
ELOC = E // NCORES  # 8 experts per core
NLOC = N // NCORES  # 512 tokens per core for the shared expert
BF16 = mybir.dt.bfloat16
FP8 = mybir.dt.float8e4
F32 = mybir.dt.float32
P = 128
KC = C // P  # 16 contraction chunks over C

_BF16_NP = ml_dtypes.bfloat16
_FP8_NP = ml_dtypes.float8_e4m3  # TRN FP8_EXP4 semantics (max 240)

GS = 64.0   # w_up gate-half quantization scale
VS = 16.0   # w_up v-half scale (keeps |h8| = |silu(g)*v*16| well under 240)
DS = 64.0   # w_down scale; yrh carries VS*DS = 1024x values, host divides
YSCALE = VS * DS

DR = mybir.MatmulPerfMode.DoubleRow

# shared-up pair order: pair i computes y chunk m=i and gate chunk m=4+i of
# the 2*HS=1024 up-projection columns
_MPAIR = [0, 4, 1, 5, 2, 6, 3, 7]


# --------------------------------------------------------------------------
# Tile tail-drain fix: this walrus build allows at most one semaphore wait per
# instruction (none on Drain). Tile's end-of-context drain carries the whole
# global clock; emit a chain of single-wait NOPs on SP instead.
# --------------------------------------------------------------------------
def _patched_drain_and_barrier(self, tick_clock, wait_clock):
    carrier = self.nc.sync.nop(nofuse=True, hint="tail_wait_0")
    wait_clock.add_sem_waits(carrier.ins, ScopedClock({None: tick_clock.global_clock}))
    si = carrier.ins.sync_info
    waits = list(si.on_wait) if si else []
    upds = list(si.on_update) if si else []
    carrier.ins.sync_info = bass_rust.SyncInfo(on_wait=waits[:1], on_update=upds)
    for i, w in enumerate(waits[1:]):
        n2 = self.nc.sync.nop(nofuse=True, hint=f"tail_wait_{i + 1}")
        n2.ins.sync_info = bass_rust.SyncInfo(on_wait=[w], on_update=[])

    self.nc.sync.drain()
    self.nc.all_engine_barrier()
    assert self.sems is not None
    popped = self.nc._tile_sem_poison_stack.pop()
    assert popped is self._sem_poison
    # sems still cleared for NEFF re-execution, but the trailing all-engine
    # barrier is redundant: engines are already drained and barriered, and
    # the gpsimd clear halts on its own before the NEFF ends
    self.nc.clear_and_free_semaphores(list(self.sems.allocated().values()))


_orig_add_instruction = TileContext._add_instruction


def _patched_add_instruction(self, inst):
    si = getattr(inst, "sync_info", None)
    if si is not None and len(si.on_wait) > 1:
        waits = list(si.on_wait)
        for w in waits[:-1]:
            nop = mybir.InstNoOp(
                name=self.nc.get_next_instruction_name(), ins=[], outs=[])
            nop.engine = inst.engine
            nop.sync_info = bass_rust.SyncInfo(on_wait=[w], on_update=[])
            _orig_add_instruction(self, nop)
        inst.sync_info = bass_rust.SyncInfo(
            on_wait=[waits[-1]], on_update=list(si.on_update))
    _orig_add_instruction(self, inst)


def _install_drain_fix():
    if getattr(TileContext, "_drain_fix_installed", False):
        return
    TileContext._drain_and_barrier = _patched_drain_and_barrier
    TileContext._add_instruction = _patched_add_instruction
    TileContext._drain_fix_installed = True


# --------------------------------------------------------------------------
# Device kernel
# --------------------------------------------------------------------------
_BUILD_CACHE = {}


def _build(caps):
    """Per-core Bass program; caps[j] = padded capacity of slot position j."""
    _install_drain_fix()
    nc = bass.Bass()

    offs = [0]
    for cp in caps:
        offs.append(offs[-1] + cp)
    S = offs[-1]

    # all HBM tensors are partition-major: [128, X] with large contiguous
    # per-partition runs so DMA packets are 4-16KB
    xdh = nc.declare_dram_parameter("xdh", [P, KC * S], FP8, isOutput=False)
    wuh = nc.declare_dram_parameter("wuh", [P, ELOC * KC * 2 * H], FP8, isOutput=False)
    wdh = nc.declare_dram_parameter("wdh", [P, ELOC * 2 * C], FP8, isOutput=False)
    xsh = nc.declare_dram_parameter("xsh", [P, KC * NLOC], BF16, isOutput=False)
    wsuh = nc.declare_dram_parameter("wsuh", [P, KC * 2 * HS], BF16, isOutput=False)
    wsdh = nc.declare_dram_parameter("wsdh", [P, 4 * C], BF16, isOutput=False)
    yrh = nc.declare_dram_parameter("yrh", [P, KC * S], BF16, isOutput=True)
    ysh = nc.declare_dram_parameter("ysh", [NLOC, C], BF16, isOutput=True)

    with TileContext(nc) as tc:
        with (
            tc.tile_pool(name="xsg_sb", bufs=4) as xsg_pool,
            tc.tile_pool(name="wsug_sb", bufs=2) as wsug_pool,
            tc.tile_pool(name="wsd_sb", bufs=1) as wsd_pool,
            tc.tile_pool(name="hsh_sb", bufs=1) as hsh_pool,
            tc.tile_pool(name="osh_sb", bufs=3) as osh_pool,
            tc.tile_pool(name="xd_sb", bufs=4) as xd_pool,
            tc.tile_pool(name="wu_sb", bufs=4) as wu_pool,
            tc.tile_pool(name="wd_sb", bufs=4) as wd_pool,
            tc.tile_pool(name="yo_sb", bufs=3) as yo_pool,
            tc.tile_pool(name="h_sb", bufs=2) as h_pool,
            tc.tile_pool(name="sg_sb", bufs=2) as sg_pool,
            tc.tile_pool(name="pu", bufs=4, space="PSUM") as pu_pool,
            tc.tile_pool(name="pd", bufs=4, space="PSUM") as pd_pool,
        ):
            xd_t = [None] * ELOC
            wu_t = [None] * ELOC
            wd_t = [None] * ELOC
            h_t = [None] * ELOC

            def load_expert(j, split=1):
                # split>1 chops xd/wu into k-range pieces so up_expert(j)'s
                # first matmuls (gated by subtile deps) start before the whole
                # expert has landed — used for expert 0 to shrink startup
                cap = caps[j]
                xd_t[j] = xd_pool.tile([P, KC, cap], FP8, tag="xd",
                                       name=f"xd_{j}")
                wu_t[j] = wu_pool.tile([P, KC, 512], FP8, tag="wu",
                                       name=f"wu_{j}")
                kq = KC // split
                for q in range(split):
                    nc.sync.dma_start(
                        out=xd_t[j][:, q * kq:(q + 1) * kq, :],
                        in_=xdh[:, KC * offs[j] + q * kq * cap:
                                KC * offs[j] + (q + 1) * kq * cap])
                    nc.sync.dma_start(
                        out=wu_t[j][:, q * kq:(q + 1) * kq, :],
                        in_=wuh[:, j * KC * 512 + q * kq * 512:
                                j * KC * 512 + (q + 1) * kq * 512])
                wd_t[j] = wd_pool.tile([P, 2, C], FP8, tag="wd",
                                       name=f"wd_{j}")
                nc.sync.dma_start(
                    out=wd_t[j][:], in_=wdh[:, j * 2 * C:(j + 1) * 2 * C])

            def up_expert(j):
                # DoubleRow fp8: each matmul contracts 2 k-tiles (256 rows);
                # psum g = GS*g_true, v = VS*v_true; silu un-scales g via
                # activation scale, h8 = silu(g)*v*VS stored fp8
                cap = caps[j]
                xd, wu = xd_t[j], wu_t[j]
                assert cap <= 512
                h8 = h_pool.tile([P, 2, cap], FP8, tag="h", name=f"h_{j}")
                for hf in range(2):
                    p_g = pu_pool.tile([P, cap], F32, space="PSUM",
                                       tag="pu", name=f"pug_{j}_{hf}")
                    p_v = pu_pool.tile([P, cap], F32, space="PSUM",
                                       tag="pu", name=f"puv_{j}_{hf}")
                    for k2 in range(KC // 2):
                        rhs = xd[:, 2 * k2:2 * k2 + 2, :]
                        nc.tensor.matmul(
                            out=p_g[:],
                            lhsT=wu[:, 2 * k2:2 * k2 + 2, hf * P:(hf + 1) * P],
                            rhs=rhs,
                            start=(k2 == 0), stop=(k2 == KC // 2 - 1),
                            perf_mode=DR)
                        nc.tensor.matmul(
                            out=p_v[:],
                            lhsT=wu[:, 2 * k2:2 * k2 + 2,
                                    2 * P + hf * P:2 * P + (hf + 1) * P],
                            rhs=rhs,
                            start=(k2 == 0), stop=(k2 == KC // 2 - 1),
                            perf_mode=DR)
                    sg = sg_pool.tile([P, cap], F32, tag="sg",
                                      name=f"sg_{j}_{hf}")
                    nc.scalar.activation(sg[:], p_g[:],
                                         mybir.ActivationFunctionType.Silu,
                                         scale=1.0 / GS)
                    nc.vector.tensor_mul(h8[:, hf, :], sg[:], p_v[:])
                h_t[j] = h8

            store_pending = []

            def flush_stores(keep=0):
                # gpsimd-queue issue; transfers start as soon as the source
                # yo's copies complete (deferral experiments all lost: the
                # DMA engines' packet round-robin shares HBM fairly enough)
                while len(store_pending) > keep:
                    j2, yo2 = store_pending.pop(0)
                    cap2 = caps[j2]
                    b2 = KC * offs[j2]
                    for hh in range(2):
                        lo = hh * 8 * cap2
                        nc.gpsimd.dma_start(
                            out=yrh[:, b2 + lo:b2 + lo + 8 * cap2],
                            in_=yo2[:, lo:lo + 8 * cap2])

            def down_expert(j, eager_store=False):
                # one DoubleRow matmul per 128-col C chunk (contraction H=256
                # = 2 k-tiles); psum->sbuf copies split vector/scalar
                cap = caps[j]
                base = KC * offs[j]
                wd, h8 = wd_t[j], h_t[j]
                yo = yo_pool.tile([P, KC * cap], BF16, tag="yo",
                                  name=f"yo_{j}")
                for cc in range(KC):
                    pd = pd_pool.tile([P, 512], F32, space="PSUM",
                                      tag="pd", name=f"pd_{j}_{cc}")
                    nc.tensor.matmul(
                        out=pd[:, 0:cap],
                        lhsT=wd[:, :, cc * P:(cc + 1) * P],
                        rhs=h8[:],
                        start=True, stop=True, perf_mode=DR)
                    # copies pace the down phase (psum reads ~1.2ns/elem
                    # on both engines): split 8/8 vector/scalar and let them
                    # drain into the next up phase via the pd buf slack
                    # (gpsimd cannot read PSUM)
                    dst = yo[:, cc * cap:(cc + 1) * cap]
                    if cc % 2 == 1:
                        nc.scalar.copy(dst, pd[:, 0:cap])
                    else:
                        nc.vector.tensor_copy(out=dst, in_=pd[:, 0:cap])
                    if cc == 0:
                        flush_stores(keep=1)
                    if eager_store and cc % 4 == 3:
                        lo = (cc - 3) * cap
                        nc.gpsimd.dma_start(
                            out=yrh[:, base + lo:base + lo + 4 * cap],
                            in_=yo[:, lo:lo + 4 * cap])
                # stores go on the (otherwise idle) gpsimd queue, deferred:
                # reads are the DMA critical path until the last expert has
                # landed, so expert j's store issues only alongside expert
                # j+1's down (flush_store below); eager for the tail
                if eager_store:
                    flush_stores()  # drain the second-to-last expert too
                else:
                    store_pending.append((j, yo))

            # ------------- shared expert helpers ----------------------------
            xs_q = [None] * 4
            wsu_p = [None] * 4

            def load_shared_x():
                for qq in range(4):
                    t = xsg_pool.tile([P, 4 * NLOC], BF16, tag="xsg",
                                      name=f"xs_{qq}")
                    nc.sync.dma_start(
                        out=t[:],
                        in_=xsh[:, qq * 4 * NLOC:(qq + 1) * 4 * NLOC])
                    xs_q[qq] = t

            def load_shared_pair(pr):
                t = wsug_pool.tile([P, KC * 256], BF16, tag="wsug",
                                   name=f"wsu_{pr}")
                nc.sync.dma_start(
                    out=t[:], in_=wsuh[:, pr * 4096:(pr + 1) * 4096])
                wsu_p[pr] = t

            def su_pass(pr):
                # shared up pair pr: y chunk and gate chunk, 16 k-tiles bf16;
                # grouped (all y then all g) rather than alternating
                ps_y = pu_pool.tile([P, NLOC], F32, space="PSUM", tag="pu",
                                    name=f"ps_y{pr}")
                ps_g = pu_pool.tile([P, NLOC], F32, space="PSUM", tag="pu",
                                    name=f"ps_g{pr}")
                wt = wsu_p[pr]
                for half, ps in ((0, ps_y), (1, ps_g)):
                    for k in range(KC):
                        lo = k * 256 + half * P
                        rhs = xs_q[k // 4][:, (k % 4) * NLOC:(k % 4 + 1) * NLOC]
                        nc.tensor.matmul(
                            out=ps[:], lhsT=wt[:, lo:lo + P], rhs=rhs,
                            start=(k == 0), stop=(k == KC - 1))
                sg = sg_pool.tile([P, NLOC], F32, tag="sg", name=f"sgs_{pr}")
                nc.scalar.activation(sg[:], ps_g[:],
                                     mybir.ActivationFunctionType.Silu)
                nc.vector.tensor_mul(
                    hsh[:, pr * NLOC:(pr + 1) * NLOC], sg[:], ps_y[:])

            def sd_pass(mt):
                # shared down token tile mt: 4 kh contraction chunks
                osh = osh_pool.tile([P, C], BF16, tag="osh",
                                    name=f"osh_{mt}")
                for ncc in range(4):
                    pd = pu_pool.tile([P, 512], F32, space="PSUM",
                                      tag="pu", name=f"pds_{mt}_{ncc}")
                    for kh in range(4):
                        nc.tensor.matmul(
                            out=pd[:],
                            lhsT=hsh[:, kh * NLOC + mt * P:
                                     kh * NLOC + (mt + 1) * P],
                            rhs=wsdh_t[:, kh * C + ncc * 512:
                                       kh * C + (ncc + 1) * 512],
                            start=(kh == 0), stop=(kh == 3))
                    dst = osh[:, ncc * 512:(ncc + 1) * 512]
                    if ncc % 2 == 1:
                        nc.scalar.copy(dst, pd[:])
                    else:
                        nc.vector.tensor_copy(out=dst, in_=pd[:])
                nc.gpsimd.dma_start(
                    out=ysh[mt * P:(mt + 1) * P, :], in_=osh[:])

            # ------------- load issue order (single sync FIFO queue) --------
            # expert 0 first (split so its first k-chunks land early), then
            # expert 1; shared tiles woven in just ahead of consumption;
            # later experts gated by pool slot release line up behind
            load_expert(0, split=4)
            load_expert(1)
            load_shared_pair(0)
            load_shared_x()
            load_expert(2)
            load_shared_pair(1)
            load_expert(3)
            load_shared_pair(2)
            load_expert(4)
            load_shared_pair(3)
            load_expert(5)
            wsdh_t = wsd_pool.tile([P, 4 * C], BF16, tag="wsd")
            nc.sync.dma_start(out=wsdh_t[:], in_=wsdh[:])
            load_expert(6)
            load_expert(7)

            hsh = hsh_pool.tile([P, 4 * NLOC], BF16, tag="hsh")

            # ------------- PE warm-up: dummy matmuls with no DMA deps -------
            # cover the preamble + expert-0 load window so the HAM clock gate
            # releases before real work starts
            nc.vector.memset(hsh[:, 0:512], 0.0)
            pwarm = pd_pool.tile([P, 512], F32, space="PSUM", tag="pd",
                                 name="pwarm")
            for wi in range(8):
                nc.tensor.matmul(out=pwarm[:], lhsT=hsh[:, 0:P],
                                 rhs=hsh[:, 0:512], start=True, stop=True)

            # ------------- compute: experts pipelined, shared interleaved ---
            # up_{j+1} issues before down_j so down's psum copies drain
            # behind matmuls; shared-up passes slot in between the early
            # experts (their weights stream while experts compute) and
            # shared-down token tiles between the late ones, so the final
            # output stores are spread instead of clustered in a tail
            up_expert(0)
            up_expert(1)
            down_expert(0)
            su_pass(0)
            up_expert(2)
            down_expert(1)
            su_pass(1)
            up_expert(3)
            down_expert(2)
            su_pass(2)
            up_expert(4)
            down_expert(3)
            su_pass(3)
            up_expert(5)
            down_expert(4)
            sd_pass(0)
            up_expert(6)
            down_expert(5)
            sd_pass(1)
            up_expert(7)
            down_expert(6)
            sd_pass(2)
            down_expert(7, eager_store=True)
            sd_pass(3)
    return nc


# --------------------------------------------------------------------------
# Host wrapper
# --------------------------------------------------------------------------
def _pm(a, nchunk):
    """[nchunk*128, X] row-major -> partition-major [128, nchunk*X]."""
    x = a.shape[1]
    return np.ascontiguousarray(
        a.reshape(nchunk, P, x).transpose(1, 0, 2)).reshape(P, nchunk * x)


def kernel(x, w_gate, w_shared_up, w_shared_down, w_up, w_down):
    x_flat = np.asarray(x, dtype=np.float32).reshape(-1, C)

    # ---- gate: sigmoid scores, top-6, normalized weights (f64 for a stable
    # ordering; ties in the fp32 reference are measure-zero) ----
    logits = x_flat.astype(np.float64) @ np.asarray(w_gate, np.float64)
    scores = 1.0 / (1.0 + np.exp(-logits))
    topk_idx = np.argsort(-scores, axis=-1, kind="stable")[:, :TOPK]
    w = np.take_along_axis(scores, topk_idx, axis=-1)
    w = w / w.sum(-1, keepdims=True)

    # ---- dispatch positions (stable within each expert, slot-major order) --
    flat_e = topk_idx.reshape(-1)
    order = np.argsort(flat_e, kind="stable")
    sorted_e = flat_e[order]
    group_start = np.searchsorted(sorted_e, np.arange(E))
    counts = np.bincount(flat_e, minlength=E)

    token_of_slot = np.arange(N * TOPK) // TOPK
    expert_slots = []   # flat (token,k) slot ids, dispatch order, per expert
    expert_tokens = []
    for e in range(E):
        slots = order[group_start[e]: group_start[e] + counts[e]]
        expert_slots.append(slots)
        expert_tokens.append(token_of_slot[slots])

    # ---- balanced expert->core assignment: sort by count desc, deal 8 at a
    # time; slot position j has the same padded cap on every core.
    # caps multiple of 16 (DoubleRow AP stride) and >=128 (FWL crossover) ----
    ranks = np.argsort(-counts, kind="stable")
    expert_of = [[int(ranks[8 * j + c]) for j in range(ELOC)]
                 for c in range(NCORES)]
    caps = tuple(
        max(128, int(-(-int(counts[ranks[8 * j]]) // 16) * 16))
        for j in range(ELOC))
    offs = [0]
    for cp in caps:
        offs.append(offs[-1] + cp)

    # ---- build per-core inputs ----
    xT8 = np.ascontiguousarray(x_flat.T).astype(_FP8_NP)     # [C, N] fp8
    xT_bf = np.ascontiguousarray(x_flat.T).astype(_BF16_NP)  # [C, N] bf16
    wsu_f = np.asarray(w_shared_up, np.float32).astype(_BF16_NP)
    wsd_f = np.asarray(w_shared_down, np.float32).astype(_BF16_NP)

    # shared-up weights in pair/k-major order (see _MPAIR)
    wr = wsu_f.reshape(KC, P, 8, P)[:, :, _MPAIR, :]        # [k, p, 8, 128]
    wsuh = np.ascontiguousarray(
        wr.reshape(KC, P, 4, 2 * P).transpose(1, 2, 0, 3)).reshape(P, KC * 2 * HS)
    wsdh = _pm(wsd_f, 4)

    # fp8 expert weights: gate cols xGS, v cols xVS; w_down xDS
    upscale = np.concatenate([np.full(H, GS, np.float32),
                              np.full(H, VS, np.float32)])

    in_maps = []
    for c in range(NCORES):
        xd_blocks = []
        wu_blocks = []
        wd_blocks = []
        for j in range(ELOC):
            e = expert_of[c][j]
            tok = expert_tokens[e]
            n = len(tok)
            blk = np.zeros((P, KC, caps[j]), dtype=_FP8_NP)
            blk[:, :, :n] = xT8[:, tok].reshape(KC, P, n).transpose(1, 0, 2)
            xd_blocks.append(blk.reshape(P, -1))
            wu_blocks.append(
                _pm(np.asarray(w_up[e], np.float32) * upscale, KC)
                .astype(_FP8_NP))
            wd_blocks.append(
                _pm(np.asarray(w_down[e], np.float32) * DS, 2)
                .astype(_FP8_NP))
        xsh = _pm(np.ascontiguousarray(
            xT_bf[:, c * NLOC:(c + 1) * NLOC]), KC)
        in_maps.append({
            "xdh": np.concatenate(xd_blocks, axis=1),
            "wuh": np.concatenate(wu_blocks, axis=1),
            "wdh": np.concatenate(wd_blocks, axis=1),
            "xsh": xsh,
            "wsuh": wsuh,
            "wsdh": wsdh,
        })

    if caps not in _BUILD_CACHE:
        _BUILD_CACHE[caps] = _build(caps)
    nc = _BUILD_CACHE[caps]

    res = run_bass_kernel_spmd(nc, in_maps, list(range(NCORES)))
    if res.exec_time_ns is not None:
        print(f"HW exec time: {res.exec_time_ns} ns", flush=True)

    # ---- host combine (unshard): gather per-slot rows, un-scale, weight, sum
    y_ts = np.empty((N * TOPK, C), dtype=np.float32)
    for c in range(NCORES):
        yr = res.results[c]["yrh"]
        for j in range(ELOC):
            e = expert_of[c][j]
            n = int(counts[e])
            seg = yr[:, KC * offs[j]:KC * offs[j] + KC * caps[j]]
            seg = seg.reshape(P, KC, caps[j])[:, :, :n]
            y_ts[expert_slots[e]] = (
                seg.transpose(2, 1, 0).reshape(n, C).astype(np.float32))
    wq = (w / YSCALE).astype(np.float32)
    routed = (y_ts.reshape(N, TOPK, C) * wq.reshape(N, TOPK, 1)).sum(axis=1)
    shared = np.concatenate(
        [r["ysh"] for r in res.results], axis=0).astype(np.float32)
    return (shared + routed).reshape(B, T, C).astype(np.float32)
